# revision 22
# baseline (speedup 1.0000x reference)
"""Trainium2 Bass kernel for nn_LocalizerBranch (embedding_lookup).

Reference computation (per full input):
    features   [B=64, S=8, D=256, F=256] f32
    key_slices [B, S] int
    W [F, F], b [F]
    valid    = 0 <= key_slices < D
    gathered = features[b, s, clip(key_slices), :]
    mean_d   = features.mean(axis=2)
    key_feat = where(valid, gathered, mean_d)        # [B, S, F]
    local    = key_feat.mean(axis=1)                 # [B, F]
    out      = relu(local @ W.T + b)                 # [B, F]

Strategy: data parallel over B, 8 batches per core, with the batch->core
assignment LOAD-BALANCED on the number of out-of-range (b, s) pairs (greedy
LPT bin-pack; outputs are un-permuted on the host).  Only invalid pairs
need their full 256 KiB D*F block (mean fallback); valid pairs need just
one 1 KiB row, so balancing the invalid count cuts the critical core's
HBM reads (e.g. 4.75 MiB -> 3.0 MiB on the reference input).

Per core the whole pre-linear computation is linear in `features`:

    localT[f, b] = sum_s blocks^T @ coeffs  +  gathered_rows^T @ vv

The invalid pairs' blocks are enumerated as 8-d-row groups spread across
all 128 partitions and fetched in `slots` rounds of one 1 MiB indirect
DMA each.  Rounds before the last are reduced over d on the DVE (strided
free-axis reduce, two f-halves) and folded into localT by one PE matmul
per 128-column f-chunk using host-built 1/(S*D) coefficients.  The LAST
round skips the DVE: its raw [rows, 8*256] tile feeds 16 small
accumulating PE matmuls (one per (g, f-half)) against the same
coefficient columns -- the identical sum in a different order -- so the
post-transfer tail is ~10x shorter and the DVE never gates the stream.
The per-pair row gather is a single 64-row indirect DMA folded in with
1/S coefficients.  The Linear runs TRANSPOSED -- outT[o, b] accumulates
one K=1 bias matmul plus four K=128 fp32 matmuls whose moving dim is only
the 8 local batches (a straight [b, o] Linear would move 256 fp32 columns
at 4 cycles/row); the host un-transposes the [2, 128, 8] result.  ReLU
runs as scalar-max on the DVE so the Activation engine stays entirely
unused (one less engine in the exit drain).  The bias travels as a 257th
row of the wt input and the aux tensor carries only coefficients, so the
setup DMAs move ~280 KiB instead of ~410 KiB.

All data-dependent values (gather indices, coefficients, batch
permutation) enter as input tensors or host-side shuffles, so one NEFF
serves all 8 cores (SPMD); only the slot COUNT is a compile-time
parameter (max over cores of ceil(n_invalid/4); builds are cached).

Toolchain notes: this container's walrus build accepts at most one sync
wait per instruction, which stock TileContext violates both at its exit
drain and in regular scheduling -- see _patch_tile_drain/_legalize_waits.
Interleaved PSUM accumulation groups on column ranges of a single PSUM
tile miscompute; separate PSUM tiles per accumulation group work.
"""

import numpy as np

B, S, D, F = 64, 8, 256, 256
NCORES = 8
BL = B // NCORES            # batches per core
NPAIR = BL * S              # (b, s) pairs per core
ROWS = NPAIR * D            # feature rows per core
P = 128                     # SBUF partitions

G = 8                       # d-rows per gather descriptor (2048 floats = 8 KiB)
NGRP = D // G               # 8-row groups per (b, s) pair block

_STATE = {}


def _patch_tile_drain():
    """This container's walrus build rejects >1 sem wait on one instruction
    ("Too many sync wait commands" on the TileContext exit Drain).  Spread
    the exit-drain waits across one nop per processor lane instead."""
    import concourse.tile as tile
    from concourse.vector_clock import ScopedClock, VectorClock

    if getattr(tile.TileContext, "_ant_drain_split", False):
        return

    def _drain_and_barrier(self, tick_clock, wait_clock):
        g = tick_clock.global_clock
        for proc in range(27):
            t = g[proc]
            if t > 0:
                vc = VectorClock()
                vc.require_at_least(proc, t)
                nop = self.nc.sync.nop(nofuse=True, hint=f"tile_exit_wait_p{proc}")
                wait_clock.add_sem_waits(nop.ins, ScopedClock({None: vc}))
        self.nc.sync.drain()
        self.nc.all_engine_barrier()
        assert self.sems is not None
        popped = self.nc._tile_sem_poison_stack.pop()
        assert popped is self._sem_poison
        self.nc.clear_and_free_semaphores(list(self.sems.allocated().values()))
        self.nc.all_engine_barrier()

    tile.TileContext._drain_and_barrier = _drain_and_barrier
    tile.TileContext._ant_drain_split = True


def _legalize_waits(nc):
    """This walrus build accepts at most one sync wait per instruction (two
    for EventSemaphore).  Tile's sem assigner emits more; split the excess
    onto same-engine NOPs inserted immediately before the instruction."""
    from concourse import mybir

    for fn in nc.m.functions:
        for blk in fn.blocks:
            new = []
            for inst in blk.instructions:
                si = inst.sync_info
                waits = list(si.on_wait) if si is not None and si.on_wait else []
                cap = 2 if isinstance(inst, mybir.InstEventSemaphore) else 1
                if len(waits) > cap:
                    keep = waits[-cap:]
                    for w in waits[:-cap]:
                        new.append(mybir.InstNoOp(
                            name=nc.get_next_instruction_name(),
                            engine=inst.engine,
                            bass_nofuse=True,
                            sync_info=mybir.SyncInfo(on_wait=[w], on_update=[]),
                        ))
                    si.on_wait = keep
                new.append(inst)
            blk.instructions[:] = new
    return nc


def _aux_cols(slots):
    """Column layout of the packed int32 aux input [P, AUXW].  (The block
    and row gather indices travel separately in `idx` so the gathers can
    start after one tiny DMA; the bias rides as a 257th row of `wt`.)"""
    c_gb = 0
    c_vv = c_gb + slots * BL
    c_ones = c_vv + BL
    return c_gb, c_vv, c_ones, c_ones + BL


def _build_v3(slots, reps=1, last_rows=P, pe_direct_last=True,
              pe_direct_all=False):
    """Sparse block gather with tiny index DMA first, per-slot accumulation
    inlined into the stream loop (two PSUM tiles -- one per localT column
    chunk), DVE reduces for all but the final slot, PE-direct accumulation
    for the final slot (short tail), row gather issued behind the block
    DMAs, bias matmul hoisted to the front of the output accumulation
    group."""
    import concourse.bass as bass
    import concourse.tile as tile
    from concourse import mybir

    _patch_tile_drain()

    c_gb, c_vv, c_ones, AUXW = _aux_cols(slots)

    nc = bass.Bass()
    f32 = mybir.dt.float32
    i32 = mybir.dt.int32
    feat = nc.dram_tensor("feat", [ROWS, F], f32, kind="ExternalInput")
    idx = nc.dram_tensor("idx", [P, slots + 1], i32, kind="ExternalInput")
    aux = nc.dram_tensor("aux", [P, AUXW], i32, kind="ExternalInput")
    wt = nc.dram_tensor("wt", [2 * P + 1, F], f32, kind="ExternalInput")
    # output leaves as [o%128, (o//128, b)] -- partition-major so each
    # partition writes one contiguous 64 B run; the host untransposes
    out = nc.dram_tensor("out", [P, 2 * BL], f32, kind="ExternalOutput")

    table = feat.rearrange("(r g) f -> r (g f)", g=G)       # [2048, 2048]

    with tile.TileContext(nc) as tc:
        with (
            tc.tile_pool(name="const", bufs=1) as cpool,
            tc.tile_pool(name="stream", bufs=min(max(slots, 2), 8)) as spool,
            tc.tile_pool(name="psum", bufs=1, space="PSUM") as ppool,
        ):
            idx_sb = cpool.tile([P, slots + 1], i32)
            nc.sync.dma_start(out=idx_sb[:], in_=idx[:])
            iblk_ap = idx_sb[:, 0:slots]
            gidx_ap = idx_sb[0:NPAIR, slots:slots + 1]

            aux_sb = cpool.tile([P, AUXW], i32)
            nc.sync.dma_start(out=aux_sb[:], in_=aux[:])
            gb_ap = aux_sb[:, c_gb:c_gb + slots * BL].bitcast(f32)
            vv_ap = aux_sb[0:NPAIR, c_vv:c_vv + BL].bitcast(f32)
            ones_ap = aux_sb[0:1, c_ones:c_ones + BL].bitcast(f32)

            wt_sb = cpool.tile([P, 2 * F], f32)
            nc.sync.dma_start(out=wt_sb[:, 0:F], in_=wt[0:P])
            nc.sync.dma_start(out=wt_sb[:, F:2 * F], in_=wt[P:2 * P])
            bias_sb = cpool.tile([1, F], f32)
            nc.sync.dma_start(out=bias_sb[:], in_=wt[2 * P:2 * P + 1])

            ps = [ppool.tile([P, BL], f32, tag=f"lt{h}", name=f"lt{h}")
                  for h in range(2)]
            gth = cpool.tile([NPAIR, F], f32)
            row_gather_done = [False]

            def do_row_gather():
                nc.gpsimd.indirect_dma_start(
                    out=gth[:], out_offset=None, in_=feat[:],
                    in_offset=bass.IndirectOffsetOnAxis(ap=gidx_ap, axis=0),
                )
                row_gather_done[0] = True

            for _rep in range(reps):   # reps>1 only for differential timing
                for s in range(slots):
                    # the final slot only carries `last_rows` real groups;
                    # don't fetch/reduce its padding (coefficients are zero)
                    pl = last_rows if s == slots - 1 else P
                    bt = spool.tile([pl, G * F], f32, tag="bt", name="bt")
                    nc.gpsimd.indirect_dma_start(
                        out=bt[:], out_offset=None, in_=table[:],
                        in_offset=bass.IndirectOffsetOnAxis(
                            ap=iblk_ap[0:pl, s:s + 1], axis=0),
                    )
                    if _rep == 0 and s == slots - 1:
                        # issue the small row gather behind the block DMAs
                        do_row_gather()
                    if (s < slots - 1 and not pe_direct_all) or not pe_direct_last:
                        part = spool.tile([P, F], f32, tag="part", name="part")
                        bt_v = bt[:].rearrange("p (g f) -> p f g", g=G)
                        for h in range(2):
                            # half-f reduce so each PE matmul starts as soon
                            # as its half is ready
                            nc.vector.reduce_sum(
                                out=part[0:pl, h * P:(h + 1) * P],
                                in_=bt_v[:, h * P:(h + 1) * P, :],
                                axis=mybir.AxisListType.X,
                            )
                            nc.tensor.matmul(
                                out=ps[h][:],
                                lhsT=part[0:pl, h * P:(h + 1) * P],
                                rhs=gb_ap[0:pl, s * BL:(s + 1) * BL],
                                start=(_rep == 0 and s == 0), stop=False,
                            )
                    else:
                        # PE-direct: fold the raw [pl, 8*256] tile into the
                        # PSUM accumulators with one matmul per (g, f-half)
                        # -- same sum as reduce-then-matmul, no DVE tail
                        for g in range(G):
                            for h in range(2):
                                nc.tensor.matmul(
                                    out=ps[h][:],
                                    lhsT=bt[0:pl, g * F + h * P:
                                            g * F + (h + 1) * P],
                                    rhs=gb_ap[0:pl, s * BL:(s + 1) * BL],
                                    start=(_rep == 0 and s == 0 and g == 0),
                                    stop=False,
                                )
            if not row_gather_done[0]:
                do_row_gather()
            for h in range(2):
                nc.tensor.matmul(
                    out=ps[h][:], lhsT=gth[:, h * P:(h + 1) * P], rhs=vv_ap,
                    start=(slots == 0), stop=True,
                )

            # transposed Linear: outT[o, b] = sum_f W.T[f, o] localT[f, b]
            # + bias[o].  All four matmuls move only 8 columns (vs two
            # 256-column fp32 matmuls the other way), exact fp32 throughout.
            poT = [ppool.tile([P, BL], f32, tag=f"ot{j}", name=f"ot{j}")
                   for j in range(2)]
            for j in range(2):
                nc.tensor.matmul(out=poT[j][:],
                                 lhsT=bias_sb[:, j * P:(j + 1) * P],
                                 rhs=ones_ap, start=True, stop=False)
            lT_sb = cpool.tile([P, 2 * BL], f32)
            # one copy per engine (DVE + GpSimd) so the halves land in
            # parallel; both engines are already in use so the exit drain
            # grows by nothing
            nc.vector.tensor_copy(lT_sb[:, 0:BL], ps[0][:])
            nc.gpsimd.tensor_copy(lT_sb[:, BL:2 * BL], ps[1][:])
            for j in range(2):
                for h in range(2):
                    nc.tensor.matmul(
                        out=poT[j][:],
                        lhsT=wt_sb[:, h * F + j * P:h * F + j * P + P],
                        rhs=lT_sb[:, h * BL:(h + 1) * BL],
                        start=False, stop=(h == 1),
                    )
            out_sb = cpool.tile([P, 2 * BL], f32)
            # ReLU as scalar-max, one half per engine; the Activation engine
            # stays entirely unused (one less engine in the exit drain)
            nc.vector.tensor_scalar_max(out_sb[:, 0:BL], poT[0][:], 0.0)
            nc.gpsimd.tensor_scalar_max(out_sb[:, BL:2 * BL], poT[1][:], 0.0)
            nc.sync.dma_start(out=out[:], in_=out_sb[:])
    return _legalize_waits(nc)


def _assign_batches(key_slices):
    """Greedy LPT bin-pack of the 64 batches onto 8 cores (8 batches each),
    balancing the number of invalid (b, s) pairs.  Returns a list of 8
    sorted batch-index lists."""
    ks = np.asarray(key_slices).astype(np.int64).reshape(B, S)
    inv_per_batch = ((ks < 0) | (ks >= D)).sum(axis=1)
    order = np.argsort(-inv_per_batch, kind="stable")
    loads = np.zeros(NCORES, np.int64)
    counts = np.zeros(NCORES, np.int64)
    groups = [[] for _ in range(NCORES)]
    for b in order:
        cands = [j for j in range(NCORES) if counts[j] < BL]
        j = min(cands, key=lambda j: loads[j])
        groups[j].append(int(b))
        loads[j] += inv_per_batch[b]
        counts[j] += 1
    return [sorted(g) for g in groups]


def make_in_maps_v3(features, key_slices, W, b):
    """Host-side sharding + coefficient prep.  Returns per-core input maps
    for the balanced batch assignment produced by _assign_batches."""
    features = np.asarray(features, dtype=np.float32)
    key_slices = np.asarray(key_slices)
    W = np.asarray(W, dtype=np.float32)
    b = np.asarray(b, dtype=np.float32)

    wt = np.concatenate(
        [np.ascontiguousarray(W.T), b.reshape(1, F)], axis=0)   # [257, F]
    p = np.arange(NPAIR)
    groups_of = _assign_batches(key_slices)
    ks_all = np.asarray(key_slices).astype(np.int64).reshape(B, S)

    cores = []
    slots = 1
    max_groups = 1
    for i in range(NCORES):
        ks = ks_all[groups_of[i]].reshape(NPAIR)
        valid = (ks >= 0) & (ks < D)
        inv = np.where(~valid)[0]
        max_groups = max(max_groups, len(inv) * NGRP)
        slots = max(slots, int(np.ceil(len(inv) * NGRP / P)))
        cores.append((ks, valid, inv))
    last_rows = max(2, max_groups - (slots - 1) * P)

    c_gb, c_vv, c_ones, AUXW = _aux_cols(slots)
    in_maps = []
    for i in range(NCORES):
        ks, valid, inv = cores[i]
        fs = features[groups_of[i]].reshape(ROWS, F)
        clip = np.clip(ks, 0, D - 1)
        idx = np.zeros((P, slots + 1), np.int32)
        aux = np.zeros((P, AUXW), np.int32)
        if len(inv):
            groups = (inv[:, None] * NGRP + np.arange(NGRP)[None, :]).reshape(-1)
        else:
            groups = np.zeros(0, np.int64)
        pad = slots * P - len(groups)
        iblk = np.concatenate([groups, np.zeros(pad, np.int64)]).astype(np.int32)
        gpair = np.concatenate([np.repeat(inv, NGRP), np.zeros(pad, np.int64)])
        real = np.concatenate([np.ones(len(groups), bool), np.zeros(pad, bool)])
        gb = np.zeros((slots * P, BL), np.float32)
        gb[np.arange(slots * P), gpair // S] = np.where(real, 1.0 / (S * D), 0.0)
        idx[:, 0:slots] = iblk.reshape(slots, P).T
        idx[0:NPAIR, slots] = (p * D + clip).astype(np.int32)
        aux[:, c_gb:c_gb + slots * BL] = (
            gb.reshape(slots, P, BL).transpose(1, 0, 2).reshape(P, slots * BL)
            .view(np.int32))
        vv = np.zeros((NPAIR, BL), np.float32)
        vv[p, p // S] = np.where(valid, 1.0 / S, 0.0)
        aux[0:NPAIR, c_vv:c_vv + BL] = vv.view(np.int32)
        aux[0, c_ones:c_ones + BL] = np.ones(BL, np.float32).view(np.int32)
        in_maps.append({"feat": fs, "idx": idx, "aux": aux, "wt": wt})
    return in_maps, slots, last_rows


def kernel(**inputs):
    from concourse.bass_utils import run_bass_kernel_spmd

    in_maps, slots, last_rows = make_in_maps_v3(
        inputs["features"], inputs["key_slices"], inputs["W"], inputs["b"])
    key = ("v4", slots, last_rows)
    if key not in _STATE:
        _STATE[key] = _build_v3(slots, last_rows=last_rows)
    res = run_bass_kernel_spmd(_STATE[key], in_maps, list(range(NCORES)))
    # un-transpose the [p, (j, b)] device output and un-permute the
    # balanced batch assignment: out[b, j*128 + p] = dev[p, j*BL + b]
    groups_of = _assign_batches(inputs["key_slices"])
    out = np.empty((B, F), np.float32)
    for i in range(NCORES):
        dev = res.results[i]["out"].reshape(P, 2, BL)       # [p, j, b]
        out[groups_of[i]] = dev.transpose(2, 1, 0).reshape(BL, F)
    return out


if __name__ == "__main__":
    d = np.load("/root/problem/ref_data.npz")
    actual = kernel(features=d["features"], key_slices=d["key_slices"],
                    W=d["W"], b=d["b"])
    expected = d["expected"]
    err = np.abs(actual - expected).max()
    print("max abs err:", err, "rel:", err / np.abs(expected).max())


# revision 24
# speedup vs baseline: 1.4608x; 1.4608x over previous
"""Trainium2 Bass kernel for nn_LocalizerBranch (embedding_lookup).

Reference computation (per full input):
    features   [B=64, S=8, D=256, F=256] f32
    key_slices [B, S] int
    W [F, F], b [F]
    valid    = 0 <= key_slices < D
    gathered = features[b, s, clip(key_slices), :]
    mean_d   = features.mean(axis=2)
    key_feat = where(valid, gathered, mean_d)        # [B, S, F]
    local    = key_feat.mean(axis=1)                 # [B, F]
    out      = relu(local @ W.T + b)                 # [B, F]

Strategy: data parallel over B, 8 batches per core, with the batch->core
assignment LOAD-BALANCED on the number of out-of-range (b, s) pairs (greedy
LPT bin-pack; outputs are un-permuted on the host).  Only invalid pairs
need their full 256 KiB D*F block (mean fallback); valid pairs need just
one 1 KiB row, so balancing the invalid count cuts the critical core's
HBM reads (e.g. 4.75 MiB -> 3.0 MiB on the reference input).

Per core the whole pre-linear computation is linear in `features`:

    localT[f, b] = sum_s blocks^T @ coeffs  +  gathered_rows^T @ vv

The invalid pairs' blocks are enumerated as 8-d-row groups spread across
all 128 partitions and fetched in `slots` rounds of one 1 MiB indirect
DMA each.  Rounds before the last are reduced over d on the DVE (strided
free-axis reduce, two f-halves) and folded into localT by one PE matmul
per 128-column f-chunk using host-built 1/(S*D) coefficients.  The LAST
round skips the DVE: its raw [rows, 8*256] tile feeds 16 small
accumulating PE matmuls (one per (g, f-half)) against the same
coefficient columns -- the identical sum in a different order -- so the
post-transfer tail is ~10x shorter and the DVE never gates the stream.
The per-pair row gather is a single 64-row indirect DMA folded in with
1/S coefficients.  The Linear runs TRANSPOSED -- outT[o, b] accumulates
one K=1 bias matmul plus four K=128 fp32 matmuls whose moving dim is only
the 8 local batches (a straight [b, o] Linear would move 256 fp32 columns
at 4 cycles/row); the host un-transposes the [2, 128, 8] result.  ReLU
runs as scalar-max on the DVE so the Activation engine stays entirely
unused (one less engine in the exit drain).  The bias travels as a 257th
row of the wt input and the aux tensor carries only coefficients, so the
setup DMAs move ~280 KiB instead of ~410 KiB.

All data-dependent values (gather indices, coefficients, batch
permutation) enter as input tensors or host-side shuffles, so one NEFF
serves all 8 cores (SPMD); only the slot COUNT is a compile-time
parameter (max over cores of ceil(n_invalid/4); builds are cached).

Toolchain notes: this container's walrus build accepts at most one sync
wait per instruction, which stock TileContext violates both at its exit
drain and in regular scheduling -- see _patch_tile_drain/_legalize_waits.
Interleaved PSUM accumulation groups on column ranges of a single PSUM
tile miscompute; separate PSUM tiles per accumulation group work.
"""

import numpy as np

B, S, D, F = 64, 8, 256, 256
NCORES = 8
BL = B // NCORES            # batches per core
NPAIR = BL * S              # (b, s) pairs per core
ROWS = NPAIR * D            # feature rows per core
P = 128                     # SBUF partitions

G = 8                       # d-rows per gather descriptor (2048 floats = 8 KiB)
NGRP = D // G               # 8-row groups per (b, s) pair block

_STATE = {}


def _patch_tile_drain():
    """This container's walrus build rejects >1 sem wait on one instruction
    ("Too many sync wait commands" on the TileContext exit Drain).  Spread
    the exit-drain waits across one nop per processor lane instead."""
    import concourse.tile as tile
    from concourse.vector_clock import ScopedClock, VectorClock

    if getattr(tile.TileContext, "_ant_drain_split", False):
        return

    def _drain_and_barrier(self, tick_clock, wait_clock):
        g = tick_clock.global_clock
        for proc in range(27):
            t = g[proc]
            if t > 0:
                vc = VectorClock()
                vc.require_at_least(proc, t)
                nop = self.nc.sync.nop(nofuse=True, hint=f"tile_exit_wait_p{proc}")
                wait_clock.add_sem_waits(nop.ins, ScopedClock({None: vc}))
        self.nc.sync.drain()
        self.nc.all_engine_barrier()
        assert self.sems is not None
        popped = self.nc._tile_sem_poison_stack.pop()
        assert popped is self._sem_poison
        self.nc.clear_and_free_semaphores(list(self.sems.allocated().values()))
        self.nc.all_engine_barrier()

    tile.TileContext._drain_and_barrier = _drain_and_barrier
    tile.TileContext._ant_drain_split = True


def _legalize_waits(nc):
    """This walrus build accepts at most one sync wait per instruction (two
    for EventSemaphore).  Tile's sem assigner emits more; split the excess
    onto same-engine NOPs inserted immediately before the instruction."""
    from concourse import mybir

    for fn in nc.m.functions:
        for blk in fn.blocks:
            new = []
            for inst in blk.instructions:
                si = inst.sync_info
                waits = list(si.on_wait) if si is not None and si.on_wait else []
                cap = 2 if isinstance(inst, mybir.InstEventSemaphore) else 1
                if len(waits) > cap:
                    keep = waits[-cap:]
                    for w in waits[:-cap]:
                        new.append(mybir.InstNoOp(
                            name=nc.get_next_instruction_name(),
                            engine=inst.engine,
                            bass_nofuse=True,
                            sync_info=mybir.SyncInfo(on_wait=[w], on_update=[]),
                        ))
                    si.on_wait = keep
                new.append(inst)
            blk.instructions[:] = new
    return nc


def _aux_cols(slots):
    """Column layout of the packed int32 aux input [P, AUXW].  (The block
    and row gather indices travel separately in `idx` so the gathers can
    start after one tiny DMA; the bias rides as a 257th row of `wt`.)"""
    c_gb = 0
    c_vv = c_gb + slots * BL
    c_ones = c_vv + BL
    return c_gb, c_vv, c_ones, c_ones + BL


def _build_v3(slots, reps=1, last_rows=P, pe_direct_last=True,
              pe_direct_all=False):
    """Sparse block gather with tiny index DMA first, per-slot accumulation
    inlined into the stream loop (two PSUM tiles -- one per localT column
    chunk), DVE reduces for all but the final slot, PE-direct accumulation
    for the final slot (short tail), row gather issued behind the block
    DMAs, bias matmul hoisted to the front of the output accumulation
    group."""
    import concourse.bass as bass
    import concourse.tile as tile
    from concourse import mybir

    _patch_tile_drain()

    c_gb, c_vv, c_ones, AUXW = _aux_cols(slots)

    nc = bass.Bass()
    f32 = mybir.dt.float32
    i32 = mybir.dt.int32
    feat = nc.dram_tensor("feat", [ROWS, F], f32, kind="ExternalInput")
    idx = nc.dram_tensor("idx", [P, slots + 1], i32, kind="ExternalInput")
    aux = nc.dram_tensor("aux", [P, AUXW], i32, kind="ExternalInput")
    wt = nc.dram_tensor("wt", [2 * P + 1, F], f32, kind="ExternalInput")
    # output leaves as [o%128, (o//128, b)] -- partition-major so each
    # partition writes one contiguous 64 B run; the host untransposes
    out = nc.dram_tensor("out", [P, 2 * BL], f32, kind="ExternalOutput")

    table = feat.rearrange("(r g) f -> r (g f)", g=G)       # [2048, 2048]

    with tile.TileContext(nc) as tc:
        with (
            tc.tile_pool(name="const", bufs=1) as cpool,
            tc.tile_pool(name="stream", bufs=min(max(slots, 2), 8)) as spool,
            tc.tile_pool(name="psum", bufs=1, space="PSUM") as ppool,
        ):
            idx_sb = cpool.tile([P, slots + 1], i32)
            nc.sync.dma_start(out=idx_sb[:], in_=idx[:])
            iblk_ap = idx_sb[:, 0:slots]
            gidx_ap = idx_sb[0:NPAIR, slots:slots + 1]

            aux_sb = cpool.tile([P, AUXW], i32)
            nc.sync.dma_start(out=aux_sb[:], in_=aux[:])
            gb_ap = aux_sb[:, c_gb:c_gb + slots * BL].bitcast(f32)
            vv_ap = aux_sb[0:NPAIR, c_vv:c_vv + BL].bitcast(f32)
            ones_ap = aux_sb[0:1, c_ones:c_ones + BL].bitcast(f32)

            wt_sb = cpool.tile([P, 2 * F], f32)
            nc.sync.dma_start(out=wt_sb[:, 0:F], in_=wt[0:P])
            nc.sync.dma_start(out=wt_sb[:, F:2 * F], in_=wt[P:2 * P])
            bias_sb = cpool.tile([1, F], f32)
            nc.sync.dma_start(out=bias_sb[:], in_=wt[2 * P:2 * P + 1])

            ps = [ppool.tile([P, BL], f32, tag=f"lt{h}", name=f"lt{h}")
                  for h in range(2)]
            gth = cpool.tile([NPAIR, F], f32)
            row_gather_done = [False]

            def do_row_gather():
                nc.gpsimd.indirect_dma_start(
                    out=gth[:], out_offset=None, in_=feat[:],
                    in_offset=bass.IndirectOffsetOnAxis(ap=gidx_ap, axis=0),
                )
                row_gather_done[0] = True

            for _rep in range(reps):   # reps>1 only for differential timing
                for s in range(slots):
                    # the final slot only carries `last_rows` real groups;
                    # don't fetch/reduce its padding (coefficients are zero)
                    pl = last_rows if s == slots - 1 else P
                    bt = spool.tile([pl, G * F], f32, tag="bt", name="bt")
                    nc.gpsimd.indirect_dma_start(
                        out=bt[:], out_offset=None, in_=table[:],
                        in_offset=bass.IndirectOffsetOnAxis(
                            ap=iblk_ap[0:pl, s:s + 1], axis=0),
                    )
                    if _rep == 0 and s == slots - 1:
                        # issue the small row gather behind the block DMAs
                        do_row_gather()
                    if (s < slots - 1 and not pe_direct_all) or not pe_direct_last:
                        part = spool.tile([P, F], f32, tag="part", name="part")
                        bt_v = bt[:].rearrange("p (g f) -> p f g", g=G)
                        for h in range(2):
                            # half-f reduce so each PE matmul starts as soon
                            # as its half is ready
                            nc.vector.reduce_sum(
                                out=part[0:pl, h * P:(h + 1) * P],
                                in_=bt_v[:, h * P:(h + 1) * P, :],
                                axis=mybir.AxisListType.X,
                            )
                            nc.tensor.matmul(
                                out=ps[h][:],
                                lhsT=part[0:pl, h * P:(h + 1) * P],
                                rhs=gb_ap[0:pl, s * BL:(s + 1) * BL],
                                start=(_rep == 0 and s == 0), stop=False,
                            )
                    else:
                        # PE-direct: fold the raw [pl, 8*256] tile into the
                        # PSUM accumulators with one matmul per (g, f-half)
                        # -- same sum as reduce-then-matmul, no DVE tail
                        for g in range(G):
                            for h in range(2):
                                nc.tensor.matmul(
                                    out=ps[h][:],
                                    lhsT=bt[0:pl, g * F + h * P:
                                            g * F + (h + 1) * P],
                                    rhs=gb_ap[0:pl, s * BL:(s + 1) * BL],
                                    start=(_rep == 0 and s == 0 and g == 0),
                                    stop=False,
                                )
            if not row_gather_done[0]:
                do_row_gather()
            for h in range(2):
                nc.tensor.matmul(
                    out=ps[h][:], lhsT=gth[:, h * P:(h + 1) * P], rhs=vv_ap,
                    start=(slots == 0), stop=True,
                )

            # transposed Linear: outT[o, b] = sum_f W.T[f, o] localT[f, b]
            # + bias[o].  All four matmuls move only 8 columns (vs two
            # 256-column fp32 matmuls the other way), exact fp32 throughout.
            poT = [ppool.tile([P, BL], f32, tag=f"ot{j}", name=f"ot{j}")
                   for j in range(2)]
            for j in range(2):
                nc.tensor.matmul(out=poT[j][:],
                                 lhsT=bias_sb[:, j * P:(j + 1) * P],
                                 rhs=ones_ap, start=True, stop=False)
            lT_sb = cpool.tile([P, 2 * BL], f32)
            # (GpSimd copies/maxes of PSUM don't compile on this walrus
            # build, so both hops stay on the DVE)
            nc.vector.tensor_copy(lT_sb[:, 0:BL], ps[0][:])
            nc.vector.tensor_copy(lT_sb[:, BL:2 * BL], ps[1][:])
            for j in range(2):
                for h in range(2):
                    nc.tensor.matmul(
                        out=poT[j][:],
                        lhsT=wt_sb[:, h * F + j * P:h * F + j * P + P],
                        rhs=lT_sb[:, h * BL:(h + 1) * BL],
                        start=False, stop=(h == 1),
                    )
            out_sb = cpool.tile([P, 2 * BL], f32)
            # ReLU as scalar-max on the DVE; keeping the whole epilogue off
            # the Activation engine drops one engine from the exit drain
            nc.vector.tensor_scalar_max(out_sb[:, 0:BL], poT[0][:], 0.0)
            nc.vector.tensor_scalar_max(out_sb[:, BL:2 * BL], poT[1][:], 0.0)
            nc.sync.dma_start(out=out[:], in_=out_sb[:])
    return _legalize_waits(nc)


def _assign_batches(key_slices):
    """Greedy LPT bin-pack of the 64 batches onto 8 cores (8 batches each),
    balancing the number of invalid (b, s) pairs.  Returns a list of 8
    sorted batch-index lists."""
    ks = np.asarray(key_slices).astype(np.int64).reshape(B, S)
    inv_per_batch = ((ks < 0) | (ks >= D)).sum(axis=1)
    order = np.argsort(-inv_per_batch, kind="stable")
    loads = np.zeros(NCORES, np.int64)
    counts = np.zeros(NCORES, np.int64)
    groups = [[] for _ in range(NCORES)]
    for b in order:
        cands = [j for j in range(NCORES) if counts[j] < BL]
        j = min(cands, key=lambda j: loads[j])
        groups[j].append(int(b))
        loads[j] += inv_per_batch[b]
        counts[j] += 1
    return [sorted(g) for g in groups]


def make_in_maps_v3(features, key_slices, W, b):
    """Host-side sharding + coefficient prep.  Returns per-core input maps
    for the balanced batch assignment produced by _assign_batches."""
    features = np.asarray(features, dtype=np.float32)
    key_slices = np.asarray(key_slices)
    W = np.asarray(W, dtype=np.float32)
    b = np.asarray(b, dtype=np.float32)

    wt = np.concatenate(
        [np.ascontiguousarray(W.T), b.reshape(1, F)], axis=0)   # [257, F]
    p = np.arange(NPAIR)
    groups_of = _assign_batches(key_slices)
    ks_all = np.asarray(key_slices).astype(np.int64).reshape(B, S)

    cores = []
    slots = 1
    max_groups = 1
    for i in range(NCORES):
        ks = ks_all[groups_of[i]].reshape(NPAIR)
        valid = (ks >= 0) & (ks < D)
        inv = np.where(~valid)[0]
        max_groups = max(max_groups, len(inv) * NGRP)
        slots = max(slots, int(np.ceil(len(inv) * NGRP / P)))
        cores.append((ks, valid, inv))
    last_rows = max(2, max_groups - (slots - 1) * P)

    c_gb, c_vv, c_ones, AUXW = _aux_cols(slots)
    in_maps = []
    for i in range(NCORES):
        ks, valid, inv = cores[i]
        fs = features[groups_of[i]].reshape(ROWS, F)
        clip = np.clip(ks, 0, D - 1)
        idx = np.zeros((P, slots + 1), np.int32)
        aux = np.zeros((P, AUXW), np.int32)
        if len(inv):
            groups = (inv[:, None] * NGRP + np.arange(NGRP)[None, :]).reshape(-1)
        else:
            groups = np.zeros(0, np.int64)
        pad = slots * P - len(groups)
        iblk = np.concatenate([groups, np.zeros(pad, np.int64)]).astype(np.int32)
        gpair = np.concatenate([np.repeat(inv, NGRP), np.zeros(pad, np.int64)])
        real = np.concatenate([np.ones(len(groups), bool), np.zeros(pad, bool)])
        gb = np.zeros((slots * P, BL), np.float32)
        gb[np.arange(slots * P), gpair // S] = np.where(real, 1.0 / (S * D), 0.0)
        idx[:, 0:slots] = iblk.reshape(slots, P).T
        idx[0:NPAIR, slots] = (p * D + clip).astype(np.int32)
        aux[:, c_gb:c_gb + slots * BL] = (
            gb.reshape(slots, P, BL).transpose(1, 0, 2).reshape(P, slots * BL)
            .view(np.int32))
        vv = np.zeros((NPAIR, BL), np.float32)
        vv[p, p // S] = np.where(valid, 1.0 / S, 0.0)
        aux[0:NPAIR, c_vv:c_vv + BL] = vv.view(np.int32)
        aux[0, c_ones:c_ones + BL] = np.ones(BL, np.float32).view(np.int32)
        in_maps.append({"feat": fs, "idx": idx, "aux": aux, "wt": wt})
    return in_maps, slots, last_rows


def kernel(**inputs):
    from concourse.bass_utils import run_bass_kernel_spmd

    in_maps, slots, last_rows = make_in_maps_v3(
        inputs["features"], inputs["key_slices"], inputs["W"], inputs["b"])
    key = ("v4", slots, last_rows)
    if key not in _STATE:
        _STATE[key] = _build_v3(slots, last_rows=last_rows)
    res = run_bass_kernel_spmd(_STATE[key], in_maps, list(range(NCORES)))
    # un-transpose the [p, (j, b)] device output and un-permute the
    # balanced batch assignment: out[b, j*128 + p] = dev[p, j*BL + b]
    groups_of = _assign_batches(inputs["key_slices"])
    out = np.empty((B, F), np.float32)
    for i in range(NCORES):
        dev = res.results[i]["out"].reshape(P, 2, BL)       # [p, j, b]
        out[groups_of[i]] = dev.transpose(2, 1, 0).reshape(BL, F)
    return out


if __name__ == "__main__":
    d = np.load("/root/problem/ref_data.npz")
    actual = kernel(features=d["features"], key_slices=d["key_slices"],
                    W=d["W"], b=d["b"])
    expected = d["expected"]
    err = np.abs(actual - expected).max()
    print("max abs err:", err, "rel:", err / np.abs(expected).max())
